# revision 20
# baseline (speedup 1.0000x reference)
"""Trainium2 Bass kernel: per-channel 8x8 box-sum pooling, stride 4 (NCHW).

Input  x: (8, 32, 512, 512) f32  ->  output (8, 32, 127, 127) f32.

Sharding: data-parallel over the batch dim — image b runs on NeuronCore b
(zero communication).

Per core, for each of the 32 channel planes (512 x 512):
  1. One 1-MiB DMA loads the plane into SBUF as [128, 4*512] laid out so
     partition p, free-chunk r holds input row h = 128*r + p.
  2. Vertical pooling runs on the tensor engine: V = Mv.T @ X accumulated
     over the four 128-row chunks into one PSUM bank, where
     Mv[h, i] = 1 iff 4i <= h < 4i+8.  V[i, w] = sum_{dh<8} x[4i+dh, w].
     The matmul runs in fp32r (1 cycle/row instead of fp32's 4): the
     weights are exactly 0/1, so only the data operand sees the reduced
     mantissa.
  3. Horizontal pooling runs on the vector engine as a pairwise tree:
     P[u] = V[2u] + V[2u+1];  Q[m] = P[2m] + P[2m+1];
     out[i, j] = Q[j] + Q[j+1].
  4. One DMA stores the [127, 127] output plane.

mode="hpool_first" swaps stages 2/3: the pairwise tree runs first on the
raw rows (exact fp32), and the vertical matmul contracts N=127 columns in
plain fp32 — no reduced-precision anywhere, at the cost of ~4x more DVE
work (still under the DMA roofline).
"""

import numpy as np

B, C, H, W = 8, 32, 512, 512
KS, ST = 8, 4
HO = (H - KS) // ST + 1  # 127
WO = (W - KS) // ST + 1  # 127
P = 128
R = H // P  # 4 row chunks per plane

MODE = "hpool_first"

_CACHE: dict = {}


def _pool_matrix(mode: str = MODE) -> np.ndarray:
    # mv[p, k*HO + i] = 1.0 iff ST*i <= h(k, p) < ST*i + KS, where chunk k
    # covers input row h = 128*k + p (row-chunk layout) or h = 256*b + 2*p + e
    # with k = 2*b + e (pair layout: two consecutive rows per partition, so
    # each input-DMA descriptor is 4 KiB instead of 2 KiB).
    mv = np.zeros((P, R * HO), dtype=np.float32)
    h = np.arange(P)[:, None]
    i = np.arange(HO)[None, :]
    for k in range(R):
        if mode.endswith("_4kb"):
            b, e = divmod(k, 2)
            hg = 256 * b + 2 * h + e
        else:
            hg = P * k + h
        mv[:, k * HO : (k + 1) * HO] = (ST * i <= hg) & (hg < ST * i + KS)
    return mv


def _build(repeat: int = 1, mode: str = MODE):
    # repeat > 1 is a dev-only timing aid: the whole per-core workload is
    # unrolled `repeat` times (re-reading the same input, overwriting the
    # same output) so wall-clock deltas isolate pure HW execution time.
    import concourse.bacc as bacc
    import concourse.mybir as mybir
    import concourse.tile as tile

    f32 = mybir.dt.float32
    f32r = mybir.dt.float32r
    mm_first = mode == "mm_first_f32r"
    xdt = f32r if mm_first else f32

    nc = bacc.Bacc("TRN2", target_bir_lowering=False, debug=False, num_devices=B)
    x_t = nc.dram_tensor("x", [C, H, W], xdt, kind="ExternalInput")
    mv_t = nc.dram_tensor("mv", [P, R * HO], xdt, kind="ExternalInput")
    out_t = nc.dram_tensor("out", [C, HO, WO], f32, kind="ExternalOutput")

    if mode.endswith("_4kb"):
        # [c, p, b, e, w]: chunk k = 2*b + e holds row h = 256*b + 2*p + e;
        # per (p, b) the (e, w) block is two consecutive DRAM rows = 4 KiB
        # contiguous, doubling the input DMA descriptor size
        x_ap = x_t.ap().rearrange("c (b p e) w -> c p b e w", p=P, e=2)
    else:
        # [c, p, r, w] view: row h = 128*r + p of plane c
        x_ap = x_t.ap().rearrange("c (r p) w -> c p r w", p=P)
    out_ap = out_t.ap()

    with tile.TileContext(nc) as tc:
        with (
            tc.tile_pool(name="consts", bufs=1) as consts,
            tc.tile_pool(name="xin", bufs=6) as xin,
            tc.tile_pool(name="vpsum", bufs=4, space="PSUM") as vpsum,
            tc.tile_pool(name="tmp", bufs=3) as tmp,
            tc.tile_pool(name="outp", bufs=4) as outp,
        ):
            mv = consts.tile([P, R * HO], xdt)
            nc.sync.dma_start(mv, mv_t.ap())
            for c in [c for _ in range(repeat) for c in range(C)]:
                xt = xin.tile([P, R * W], xdt)
                if mode.endswith("_4kb"):
                    nc.sync.dma_start(
                        xt[:].rearrange("p (b e w) -> p b e w", b=2, e=2), x_ap[c]
                    )
                else:
                    nc.sync.dma_start(
                        xt[:].rearrange("p (r w) -> p r w", r=R), x_ap[c]
                    )
                if mode == "hybrid":
                    # balance engines with exact fp32 everywhere: planes with
                    # (c % 2 == 0) take the PE-heavy path (plain-fp32 N=512
                    # matmul first), the rest take the DVE-heavy path
                    # (pairwise tree first, N=127 matmul second)
                    plane_mm_first = c % 2 == 0
                else:
                    plane_mm_first = mm_first
                if plane_mm_first:
                    v = vpsum.tile([HO, W], f32)
                    for r in range(R):
                        nc.tensor.matmul(
                            v,
                            mv[:, r * HO : (r + 1) * HO],
                            xt[:, r * W : (r + 1) * W],
                            start=(r == 0),
                            stop=(r == R - 1),
                        )
                    v2 = v[:].rearrange("i (u two) -> i u two", two=2)
                    a = tmp.tile([HO, W // 2], f32)
                    nc.vector.tensor_copy(a, v2[:, :, 0])
                    p2 = tmp.tile([HO, W // 2], f32)
                    nc.vector.tensor_add(p2, v2[:, :, 1], a)
                    p2v = p2[:].rearrange("i (u two) -> i u two", two=2)
                    p4 = tmp.tile([HO, W // 4], f32)
                    nc.vector.tensor_add(p4, p2v[:, :, 0], p2v[:, :, 1])
                    o = outp.tile([HO, WO], f32)
                    nc.vector.tensor_add(o, p4[:, 0:WO], p4[:, 1 : WO + 1])
                else:
                    x2 = xt[:].rearrange("p (r u two) -> p r u two", r=R, two=2)
                    p2 = tmp.tile([P, R * (W // 2)], f32)
                    p2w = p2[:].rearrange("p (r u) -> p r u", r=R)
                    nc.vector.tensor_add(p2w, x2[:, :, :, 0], x2[:, :, :, 1])
                    p2v = p2[:].rearrange("p (r m two) -> p r m two", r=R, two=2)
                    p4 = tmp.tile([P, R * (W // 4)], f32)
                    p4w = p4[:].rearrange("p (r m) -> p r m", r=R)
                    nc.vector.tensor_add(p4w, p2v[:, :, :, 0], p2v[:, :, :, 1])
                    p4v = p4[:].rearrange("p (r m) -> p r m", r=R)
                    hp = tmp.tile([P, R * WO], f32)
                    hpw = hp[:].rearrange("p (r j) -> p r j", r=R)
                    nc.vector.tensor_add(
                        hpw, p4v[:, :, 0:WO], p4v[:, :, 1 : WO + 1]
                    )
                    v = vpsum.tile([HO, WO], f32)
                    for r in range(R):
                        nc.tensor.matmul(
                            v,
                            mv[:, r * HO : (r + 1) * HO],
                            hp[:, r * WO : (r + 1) * WO],
                            start=(r == 0),
                            stop=(r == R - 1),
                        )
                    o = outp.tile([HO, WO], f32)
                    nc.vector.tensor_copy(o, v)
                nc.sync.dma_start(out_ap[c], o)
    nc.compile()
    return nc


def kernel(x: np.ndarray) -> np.ndarray:
    from concourse import bass_utils

    nc = _CACHE.get("nc")
    if nc is None:
        nc = _CACHE["nc"] = _build()
    x = np.ascontiguousarray(np.asarray(x, dtype=np.float32))
    assert x.shape == (B, C, H, W)
    mv = _pool_matrix()
    in_maps = [{"x": x[b], "mv": mv} for b in range(B)]
    res = bass_utils.run_bass_kernel_spmd(nc, in_maps, core_ids=list(range(B)))
    return np.stack([res.results[b]["out"] for b in range(B)], axis=0)
